# revision 1
# baseline (speedup 1.0000x reference)
"""TRN2 Bass kernel for nn_Attention_24309514895857.

Multi-head attention (16 heads, dim_head 128, d_model 2048, b=2, n=2048) with
rotary embedding, sharded tensor-parallel over 8 NeuronCores: 2 heads per core.
Each core computes q/k/v projections for its heads, rotary, softmax attention,
and its partial contribution to the output projection (row-parallel Wo). The
host sums the 8 partials (the row-parallel unshard) and adds the bias.

All matmuls run in float32r (TF32-like single-pass fp32, full PE rate).
Everything on-device is feature-major ("transposed") so no transposes are
needed: x arrives as xT (d_model, tokens), q/k live as (dim_head, tokens),
attention scores as (k_tok, q_tok), output partial leaves as yT (d_model, tok).

rotate_half is a fixed pair-swap permutation of the dim_head axis -> done with
a 128x128 permutation matmul on the PE; the sign and the 1/sqrt(d) scale are
folded into host-precomputed sin/cos tables and Wq respectively.

Softmax skips the max-subtraction (logits are ~N(0,1) here; exp is safe) so
the denominator comes from an all-ones matmul that also broadcasts the sums
across all 128 partitions for the normalization divide.
"""

import numpy as np

HEADS = 16
DH = 128          # dim_head
D = 2048          # d_model
B = 2
N = 2048          # seq len
TOK = B * N       # 4096 flattened tokens
NCORES = 8
HPC = HEADS // NCORES   # 2 heads per core
INC = HPC * DH          # 256 inner cols per core
KCH = D // 128          # 16 model-dim chunks
TC = TOK // 512         # 8 token chunks of 512
KT = N // 128           # 16 k-token chunks of 128 per batch
SCALE = DH ** -0.5

_CACHE = {}

# DVE stream_shuffle mask: swap adjacent lane pairs within each 32-lane group
SWAP_MASK = []
for _i in range(16):
    SWAP_MASK += [2 * _i + 1, 2 * _i]


def _build():
    import concourse.bacc as bacc
    import concourse.tile as tile
    from concourse import mybir

    f32 = mybir.dt.float32
    f32r = mybir.dt.float32r

    nc = bacc.Bacc("TRN2", target_bir_lowering=False, debug=False,
                   num_devices=NCORES)

    xt_d = nc.dram_tensor("xt", [D, TOK], f32, kind="ExternalInput").ap()
    wq_d = nc.dram_tensor("wq", [D, INC], f32, kind="ExternalInput").ap()
    wk_d = nc.dram_tensor("wk", [D, INC], f32, kind="ExternalInput").ap()
    wv_d = nc.dram_tensor("wv", [D, INC], f32, kind="ExternalInput").ap()
    wo_d = nc.dram_tensor("wo", [INC, D], f32, kind="ExternalInput").ap()
    cos_d = nc.dram_tensor("cost", [DH, N], f32, kind="ExternalInput").ap()
    sin_d = nc.dram_tensor("sint", [DH, N], f32, kind="ExternalInput").ap()
    bo_d = nc.dram_tensor("bo8t", [128, KCH], f32, kind="ExternalInput").ap()
    yt_d = nc.dram_tensor("yt", [D, TOK], f32, kind="ExternalOutput").ap()

    xt_r = xt_d.bitcast(f32r).rearrange("(k p) t -> p k t", p=128)
    wq_r = wq_d.bitcast(f32r).rearrange("(k p) j -> p k j", p=128)
    wk_r = wk_d.bitcast(f32r).rearrange("(k p) j -> p k j", p=128)
    wv_r = wv_d.bitcast(f32r).rearrange("(k p) j -> p k j", p=128)
    wo_r = wo_d.bitcast(f32r).rearrange("(h p) m -> p h m", p=128)

    with tile.TileContext(nc) as tc:
        import contextlib
        with contextlib.ExitStack() as stack:
            glob = stack.enter_context(tc.tile_pool(name="glob", bufs=1))
            qkv = stack.enter_context(tc.tile_pool(name="qkv", bufs=1))
            rot = stack.enter_context(tc.tile_pool(name="rot", bufs=1))

            onesf = glob.tile([128, 128], f32)
            nc.vector.memset(onesf, 1.0)
            ones = glob.tile([128, 128], f32r)
            nc.vector.tensor_copy(out=ones, in_=onesf)
            bo8 = glob.tile([128, KCH], f32)
            nc.scalar.dma_start(out=bo8, in_=bo_d)

            # persistent per-head activations (feature-major), split per
            # batch so phase B's first reads only depend on that batch's
            # phase-A writes (Tile deps are per-tile)
            qrt = [[qkv.tile([DH, N], f32r, name=f"qrt{h}b{b}")
                    for b in range(B)] for h in range(HPC)]
            krt = [[qkv.tile([DH, N], f32r, name=f"krt{h}b{b}")
                    for b in range(B)] for h in range(HPC)]
            vnat = [[qkv.tile([128, KT, DH], f32r, name=f"vnat{h}b{b}")
                     for b in range(B)] for h in range(HPC)]

            # HAM warm-up: ~5us of free matmuls while the first DMAs land,
            # so real matmuls start at 2.4GHz instead of the cold 1.2GHz
            with tc.tile_pool(name="pwarm", bufs=1, space="PSUM") as pw:
                wps = pw.tile([128, 128], f32)
                NWARM = 28
                for i in range(NWARM):
                    nc.tensor.matmul(wps, ones[:], ones[:],
                                     start=(i == 0), stop=(i == NWARM - 1))

            # ---------------- Phase A: projections + rotary ----------------
            with contextlib.ExitStack() as sa:
                wpool = sa.enter_context(tc.tile_pool(name="wpool", bufs=1))
                apool = sa.enter_context(tc.tile_pool(name="apool", bufs=1))
                # per-k weight tiles, DMA'd just-in-time inside tc=0's k-loop
                # so the first matmul starts ~1us in instead of waiting 35us
                # for monolithic weight loads
                wq_t = [wpool.tile([128, INC], f32r, name=f"wq{k}")
                        for k in range(KCH)]
                wk_t = [wpool.tile([128, INC], f32r, name=f"wk{k}")
                        for k in range(KCH)]
                wv_t = [wpool.tile([128, INC], f32r, name=f"wv{k}")
                        for k in range(KCH)]
                psA = sa.enter_context(tc.tile_pool(name="psA", bufs=1,
                                                    space="PSUM"))
                cost = apool.tile([DH, N], f32)
                sint = apool.tile([DH, N], f32)
                costL = rot.tile([DH, 512], f32)
                sintL = rot.tile([DH, 512], f32)
                # first two x chunks prefetched on the scalar queue so the
                # first matmul isn't waiting behind the weight stream
                xt_pre = []
                for k in range(2):
                    xtp = apool.tile([128, 512], f32r, name="xt", tag="xt",
                                     bufs=8)
                    nc.scalar.dma_start(out=xtp, in_=xt_r[:, k, 0:512])
                    xt_pre.append(xtp)
                # all weight/const DMAs upfront on the scalar queue, k-interleaved
                # so the tc=0 k-loop's weights arrive in consumption order
                for k in range(KCH):
                    nc.scalar.dma_start(out=wq_t[k], in_=wq_r[:, k, :])
                    nc.scalar.dma_start(out=wk_t[k], in_=wk_r[:, k, :])
                    nc.sync.dma_start(out=wv_t[k], in_=wv_r[:, k, :])
                nc.scalar.dma_start(out=cost, in_=cos_d)
                nc.scalar.dma_start(out=sint, in_=sin_d)

                for t in range(TC):
                    tok0 = t * 512
                    tb = tok0 // N
                    bo0 = tok0 - tb * N
                    if t == TC - 1:
                        lsl = slice((t % (N // 512)) * 512,
                                    (t % (N // 512)) * 512 + 512)
                        nc.scalar.copy(out=costL, in_=cost[:, lsl])
                        nc.scalar.copy(out=sintL, in_=sint[:, lsl])
                    nsl = slice((t % (N // 512)) * 512,
                                (t % (N // 512)) * 512 + 512)
                    qp = [psA.tile([128, 512], f32, name=f"qp{h}", tag=f"qp{h}")
                          for h in range(HPC)]
                    kp = [psA.tile([128, 512], f32, name=f"kp{h}", tag=f"kp{h}")
                          for h in range(HPC)]
                    vp = [psA.tile([128, INC], f32, name=f"vp{s}",
                                   tag=f"vp{s}", bufs=1) for s in range(4)]
                    for k in range(KCH):
                        if t == 0 and k < 2:
                            xt = xt_pre[k]
                        else:
                            xt = apool.tile([128, 512], f32r, name="xt",
                                            tag="xt", bufs=8)
                            nc.sync.dma_start(
                                out=xt, in_=xt_r[:, k, tok0:tok0 + 512])
                        for h in range(HPC):
                            nc.tensor.matmul(
                                qp[h], wq_t[k][:, h * DH:(h + 1) * DH], xt[:],
                                start=(k == 0), stop=(k == KCH - 1))
                            nc.tensor.matmul(
                                kp[h], wk_t[k][:, h * DH:(h + 1) * DH], xt[:],
                                start=(k == 0), stop=(k == KCH - 1))
                        for sub in range(4):
                            nc.tensor.matmul(
                                vp[sub],
                                xt[:, sub * 128:(sub + 1) * 128],
                                wv_t[k][:],
                                start=(k == 0), stop=(k == KCH - 1))
                    # v psum -> token-major SBUF (DVE, emitted first so the
                    # vp banks free early for the next tc iteration)
                    for sub in range(4):
                        chunk = (t % 4) * 4 + sub
                        for h in range(HPC):
                            nc.vector.tensor_copy(
                                out=vnat[h][tb][:, chunk, :],
                                in_=vp[sub][:, h * DH:(h + 1) * DH])
                    # rotary for q and k of both heads; rotate_half's pair
                    # swap is a single DVE stream_shuffle (32-lane pair swap,
                    # uniform across quadrants); sign lives in sint
                    for h in range(HPC):
                        for (pp, dst) in ((qp[h], qrt[h][tb]),
                                          (kp[h], krt[h][tb])):
                            sb = rot.tile([128, 512], f32r, name="rsb",
                                           tag="rsb", bufs=4)
                            nc.scalar.copy(out=sb, in_=pp)
                            sbs = rot.tile([128, 512], f32, name="sbs",
                                            tag="sbs", bufs=2)
                            nc.vector.stream_shuffle(
                                out=sbs[:], in_=sb[:].bitcast(f32),
                                mask=SWAP_MASK)
                            cs = cost[:, nsl] if t < TC - 1 else costL[:]
                            sn = sint[:, nsl] if t < TC - 1 else sintL[:]
                            t1 = rot.tile([128, 512], f32, name="t1",
                                          tag="t1", bufs=3)
                            nc.vector.tensor_mul(
                                t1[:], sb[:].bitcast(f32), cs)
                            t2 = rot.tile([128, 512], f32, name="t2",
                                          tag="t2", bufs=3)
                            nc.vector.tensor_mul(t2[:], sbs[:], sn)
                            nc.vector.tensor_add(
                                dst[:, bo0:bo0 + 512], t1[:], t2[:])

            # ---------------- Phase B+C: attention + output proj -----------
            with contextlib.ExitStack() as sb_:
                bpool = sb_.enter_context(tc.tile_pool(name="bpool", bufs=1))
                psD = sb_.enter_context(tc.tile_pool(name="psD", bufs=1,
                                                     space="PSUM"))

                wo_t = [bpool.tile([DH, D], f32r, name=f"wo{h}")
                        for h in range(HPC)]
                for h in range(HPC):
                    nc.sync.dma_start(out=wo_t[h], in_=wo_r[:, h, :])

                for qc in range(TC):
                    q0 = qc * 512
                    b = q0 // N
                    outs = []
                    for h in range(HPC):
                        # scores^T: [k_tok, q_tok], exp'd, in 16 chunks
                        exps = []
                        for kt in range(KT):
                            dp = psD.tile([128, 512], f32, name="dp",
                                          tag="dp", bufs=4)
                            nc.tensor.matmul(
                                dp,
                                krt[h][b][:, kt * 128:(kt + 1) * 128],
                                qrt[h][b][:, q0 - b * N:q0 - b * N + 512],
                                start=True, stop=True)
                            ex = bpool.tile([128, 512], f32r, name="ex",
                                            tag="ex", bufs=22)
                            nc.scalar.activation(
                                out=ex, in_=dp[:],
                                func=mybir.ActivationFunctionType.Exp)
                            exps.append(ex)
                        sp = psD.tile([128, 512], f32, name="sp",
                                      tag="sp", bufs=1)
                        ap = psD.tile([128, 512], f32, name="ap",
                                      tag="ap", bufs=1)
                        for kt in range(KT):
                            nc.tensor.matmul(sp, ones[:], exps[kt][:],
                                             start=(kt == 0),
                                             stop=(kt == KT - 1))
                            nc.tensor.matmul(ap, vnat[h][b][:, kt, :],
                                             exps[kt][:],
                                             start=(kt == 0),
                                             stop=(kt == KT - 1))
                        rscr = bpool.tile([128, 512], f32, name="rscr",
                                          tag="rscr", bufs=1)
                        rcp = bpool.tile([128, 512], f32, name="rcp",
                                         tag="rcp", bufs=2)
                        nc.vector.reciprocal_approx_accurate(
                            out=rcp[:], in_=sp[:], scratch=rscr[:])
                        ot = bpool.tile([128, 512], f32r, name=f"ot{h}",
                                        tag=f"ot{h}", bufs=2)
                        nc.vector.tensor_mul(ot[:], ap[:], rcp[:])
                        outs.append(ot)
                    # output projection for this q-chunk; m-pairs with all
                    # h0 matmuls first so the h1 normalization latency hides
                    for m0 in range(0, KCH, 2):
                        yps = [psD.tile([128, 512], f32, name=f"yp{j}",
                                        tag=f"yp{j}", bufs=1)
                               for j in range(2)]
                        for h in range(HPC):
                            for j in range(2):
                                m = m0 + j
                                nc.tensor.matmul(
                                    yps[j], wo_t[h][:, m * 128:(m + 1) * 128],
                                    outs[h][:],
                                    start=(h == 0), stop=(h == HPC - 1))
                        for j in range(2):
                            m = m0 + j
                            ysb = bpool.tile([128, 512], f32, name="ysb",
                                             tag="ysb", bufs=4)
                            nc.vector.tensor_scalar_add(ysb[:], yps[j][:],
                                                        bo8[:, m:m + 1])
                            eng = nc.sync if m % 2 == 0 else nc.scalar
                            eng.dma_start(
                                out=yt_d[m * 128:(m + 1) * 128, q0:q0 + 512],
                                in_=ysb[:])

    nc.compile()
    return nc


def _host_prep(x, rotary_emb, Wq, Wkv, Wo, bo):
    x = np.asarray(x, dtype=np.float32)
    rotary_emb = np.asarray(rotary_emb, dtype=np.float32)
    Wq = np.asarray(Wq, dtype=np.float32)
    Wkv = np.asarray(Wkv, dtype=np.float32)
    Wo = np.asarray(Wo, dtype=np.float32)
    bo = np.asarray(bo, dtype=np.float32)

    xt = np.ascontiguousarray(x.reshape(TOK, D).T)
    cost = np.ascontiguousarray(np.cos(rotary_emb).T)
    sgn = np.where(np.arange(DH) % 2 == 0, -1.0, 1.0).astype(np.float32)
    sint = np.ascontiguousarray((np.sin(rotary_emb) * sgn).T)
    bo8t = np.ascontiguousarray((bo / NCORES).reshape(KCH, 128).T)

    in_maps = []
    for c in range(NCORES):
        sl = slice(c * INC, (c + 1) * INC)
        in_maps.append({
            "xt": xt,
            "wq": np.ascontiguousarray(Wq[:, sl] * SCALE),
            "wk": np.ascontiguousarray(Wkv[:, sl]),
            "wv": np.ascontiguousarray(Wkv[:, D + c * INC:D + (c + 1) * INC]),
            "wo": np.ascontiguousarray(Wo[sl, :]),
            "cost": cost,
            "sint": sint,
            "bo8t": bo8t,
        })
    return in_maps


def _get_nc():
    if "nc" not in _CACHE:
        _CACHE["nc"] = _build()
    return _CACHE["nc"]


def run_sharded(in_maps, trace=False, tmpdir=None):
    from concourse.bass_utils import run_bass_kernel_spmd
    nc = _get_nc()
    return run_bass_kernel_spmd(nc, in_maps, list(range(NCORES)),
                                trace=trace, tmpdir=tmpdir)


def kernel(x, rotary_emb, Wq, Wkv, Wo, bo):
    in_maps = _host_prep(x, rotary_emb, Wq, Wkv, Wo, bo)
    res = run_sharded(in_maps)
    yt = res.results[0]["yt"].astype(np.float64)
    for c in range(1, NCORES):
        yt += res.results[c]["yt"]
    return np.ascontiguousarray(yt.T).reshape(B, N, D).astype(np.float32)



# revision 2
# speedup vs baseline: 1.0509x; 1.0509x over previous
"""TRN2 Bass kernel for nn_Attention_24309514895857.

Multi-head attention (16 heads, dim_head 128, d_model 2048, b=2, n=2048) with
rotary embedding, sharded tensor-parallel over 8 NeuronCores: 2 heads per core.
Each core computes q/k/v projections for its heads, rotary, softmax attention,
and its partial contribution to the output projection (row-parallel Wo). The
host sums the 8 partials (the row-parallel unshard) and adds the bias.

Perf structure (v2):
- Projections run in bf16 (x and Wq/Wk/Wv are host-cast): halves the phase-A
  HBM stream so the PE never starves at t=0 (keeps HAM at 2.4GHz). PSUM
  accumulation is fp32, so everything downstream is unchanged.
- Softmax denominator: instead of 16 accumulating ones-matmuls per
  (q-chunk, head) on the PE (54us of PE time), the exp'd score tiles are
  pairwise-reduced on DVE (self-folds, bf16 2x mode) + Pool (tree adds),
  leaving only TWO ones-matmuls per (q-chunk, head) for the final
  128-partition reduction + broadcast.
- exp runs on ACT over [128,1024] two-bank PSUM tiles (half the instruction
  overhead) and writes bf16; the attn@v matmuls consume the bf16 exps.
- The Wo matmuls for q-chunk qc are deferred and interleaved into the
  attention matmuls of qc+1, so the PE never waits on the softmax
  normalization (DVE reciprocal) latency.
- Output bias is applied on the host; PSUM->SBUF output copies run on DVE
  and stream out on the sync DMA queue.
"""

import numpy as np

HEADS = 16
DH = 128          # dim_head
D = 2048          # d_model
B = 2
N = 2048          # seq len
TOK = B * N       # 4096 flattened tokens
NCORES = 8
HPC = HEADS // NCORES   # 2 heads per core
INC = HPC * DH          # 256 inner cols per core
KCH = D // 128          # 16 model-dim chunks
TC = TOK // 512         # 8 token chunks of 512
KT = N // 128           # 16 k-token chunks of 128 per batch
KP = KT // 2            # 8 k-token chunk PAIRS (dp tiles span 2 chunks)
SCALE = DH ** -0.5

_CACHE = {}

# DVE stream_shuffle mask: swap adjacent lane pairs within each 32-lane group
SWAP_MASK = []
for _i in range(16):
    SWAP_MASK += [2 * _i + 1, 2 * _i]


def _build():
    import concourse.bacc as bacc
    import concourse.tile as tile
    from concourse import mybir

    f32 = mybir.dt.float32
    f32r = mybir.dt.float32r
    bf16 = mybir.dt.bfloat16

    nc = bacc.Bacc("TRN2", target_bir_lowering=False, debug=False,
                   num_devices=NCORES)

    xt_d = nc.dram_tensor("xt", [D, TOK], bf16, kind="ExternalInput").ap()
    wq_d = nc.dram_tensor("wq", [D, INC], bf16, kind="ExternalInput").ap()
    wk_d = nc.dram_tensor("wk", [D, INC], bf16, kind="ExternalInput").ap()
    wv_d = nc.dram_tensor("wv", [D, INC], bf16, kind="ExternalInput").ap()
    wo_d = nc.dram_tensor("wo", [INC, D], f32, kind="ExternalInput").ap()
    cos_d = nc.dram_tensor("cost", [DH, N], f32, kind="ExternalInput").ap()
    sin_d = nc.dram_tensor("sint", [DH, N], f32, kind="ExternalInput").ap()
    yt_d = nc.dram_tensor("yt", [D, TOK], f32, kind="ExternalOutput").ap()

    xt_r = xt_d.rearrange("(k p) t -> p k t", p=128)
    wq_r = wq_d.rearrange("(k p) j -> p k j", p=128)
    wk_r = wk_d.rearrange("(k p) j -> p k j", p=128)
    wv_r = wv_d.rearrange("(k p) j -> p k j", p=128)
    wo_r = wo_d.bitcast(f32r).rearrange("(h p) m -> p h m", p=128)

    with tile.TileContext(nc) as tc:
        import contextlib
        with contextlib.ExitStack() as stack:
            glob = stack.enter_context(tc.tile_pool(name="glob", bufs=1))
            qkv = stack.enter_context(tc.tile_pool(name="qkv", bufs=1))
            rot = stack.enter_context(tc.tile_pool(name="rot", bufs=1))

            onesf = glob.tile([128, 128], f32)
            nc.vector.memset(onesf, 1.0)
            ones = glob.tile([128, 128], f32r)
            nc.vector.tensor_copy(out=ones, in_=onesf)
            onesb = glob.tile([128, 128], bf16)
            nc.vector.tensor_copy(out=onesb, in_=onesf)

            # persistent per-head activations, split per batch so phase B's
            # reads only depend on that batch's phase-A writes
            qrt = [[qkv.tile([DH, N], f32r, name=f"qrt{h}b{b}")
                    for b in range(B)] for h in range(HPC)]
            krt = [[qkv.tile([DH, N], f32r, name=f"krt{h}b{b}")
                    for b in range(B)] for h in range(HPC)]
            vnat = [[qkv.tile([128, KT, DH], bf16, name=f"vnat{h}b{b}")
                     for b in range(B)] for h in range(HPC)]

            # HAM warm-up: free matmuls while the first DMAs land, so real
            # matmuls start at 2.4GHz instead of the cold 1.2GHz
            with tc.tile_pool(name="pwarm", bufs=1, space="PSUM") as pw:
                wps = pw.tile([128, 128], f32)
                NWARM = 28
                for i in range(NWARM):
                    nc.tensor.matmul(wps, ones[:], ones[:],
                                     start=(i == 0), stop=(i == NWARM - 1))

            # ---------------- Phase A: projections + rotary ----------------
            with contextlib.ExitStack() as sa:
                wpool = sa.enter_context(tc.tile_pool(name="wpool", bufs=1))
                apool = sa.enter_context(tc.tile_pool(name="apool", bufs=1))
                wq_t = [wpool.tile([128, INC], bf16, name=f"wq{k}")
                        for k in range(KCH)]
                wk_t = [wpool.tile([128, INC], bf16, name=f"wk{k}")
                        for k in range(KCH)]
                wv_t = [wpool.tile([128, INC], bf16, name=f"wv{k}")
                        for k in range(KCH)]
                psA = sa.enter_context(tc.tile_pool(name="psA", bufs=1,
                                                    space="PSUM"))
                cost = apool.tile([DH, N], f32)
                sint = apool.tile([DH, N], f32)
                costL = rot.tile([DH, 512], f32)
                sintL = rot.tile([DH, 512], f32)
                # weights + rotary tables on the scalar DMA queue in
                # consumption order; x tiles stream JIT on the sync queue
                for k in range(KCH):
                    nc.scalar.dma_start(out=wq_t[k], in_=wq_r[:, k, :])
                    nc.scalar.dma_start(out=wk_t[k], in_=wk_r[:, k, :])
                    nc.scalar.dma_start(out=wv_t[k], in_=wv_r[:, k, :])
                    if k == 10:
                        nc.scalar.dma_start(out=cost, in_=cos_d)
                        nc.scalar.dma_start(out=sint, in_=sin_d)

                for t in range(TC):
                    tok0 = t * 512
                    tb = tok0 // N
                    bo0 = tok0 - tb * N
                    if t == TC - 1:
                        lsl = slice((t % (N // 512)) * 512,
                                    (t % (N // 512)) * 512 + 512)
                        nc.scalar.copy(out=costL, in_=cost[:, lsl])
                        nc.scalar.copy(out=sintL, in_=sint[:, lsl])
                    nsl = slice((t % (N // 512)) * 512,
                                (t % (N // 512)) * 512 + 512)
                    qp = [psA.tile([128, 512], f32, name=f"qp{h}", tag=f"qp{h}")
                          for h in range(HPC)]
                    kp = [psA.tile([128, 512], f32, name=f"kp{h}", tag=f"kp{h}")
                          for h in range(HPC)]
                    vp = [psA.tile([128, INC], f32, name=f"vp{s}",
                                   tag=f"vp{s}", bufs=1) for s in range(4)]
                    for k in range(KCH):
                        xt = apool.tile([128, 512], bf16, name="xt",
                                        tag="xt", bufs=8)
                        nc.sync.dma_start(
                            out=xt, in_=xt_r[:, k, tok0:tok0 + 512])
                        for h in range(HPC):
                            nc.tensor.matmul(
                                qp[h], wq_t[k][:, h * DH:(h + 1) * DH], xt[:],
                                start=(k == 0), stop=(k == KCH - 1))
                            nc.tensor.matmul(
                                kp[h], wk_t[k][:, h * DH:(h + 1) * DH], xt[:],
                                start=(k == 0), stop=(k == KCH - 1))
                        for sub in range(4):
                            nc.tensor.matmul(
                                vp[sub],
                                xt[:, sub * 128:(sub + 1) * 128],
                                wv_t[k][:],
                                start=(k == 0), stop=(k == KCH - 1))
                    # v psum -> token-major bf16 SBUF on ACT (frees vp banks;
                    # keeps DVE free for rotary)
                    for sub in range(4):
                        chunk = (t % 4) * 4 + sub
                        for h in range(HPC):
                            nc.scalar.copy(
                                out=vnat[h][tb][:, chunk, :],
                                in_=vp[sub][:, h * DH:(h + 1) * DH])
                    # rotary for q and k of both heads (DVE): rotate_half's
                    # pair swap is one stream_shuffle; sign lives in sint
                    for h in range(HPC):
                        for (pp, dst) in ((qp[h], qrt[h][tb]),
                                          (kp[h], krt[h][tb])):
                            sb = rot.tile([128, 512], f32r, name="rsb",
                                          tag="rsb", bufs=4)
                            nc.scalar.copy(out=sb, in_=pp)
                            sbs = rot.tile([128, 512], f32, name="sbs",
                                           tag="sbs", bufs=2)
                            nc.vector.stream_shuffle(
                                out=sbs[:], in_=sb[:].bitcast(f32),
                                mask=SWAP_MASK)
                            cs = cost[:, nsl] if t < TC - 1 else costL[:]
                            sn = sint[:, nsl] if t < TC - 1 else sintL[:]
                            t1 = rot.tile([128, 512], f32, name="t1",
                                          tag="t1", bufs=3)
                            nc.vector.tensor_mul(
                                t1[:], sb[:].bitcast(f32), cs)
                            t2 = rot.tile([128, 512], f32, name="t2",
                                          tag="t2", bufs=3)
                            nc.vector.tensor_mul(t2[:], sbs[:], sn)
                            nc.vector.tensor_add(
                                dst[:, bo0:bo0 + 512], t1[:], t2[:])

            # ---------------- Phase B+C: attention + output proj -----------
            with contextlib.ExitStack() as sb_:
                bpool = sb_.enter_context(tc.tile_pool(name="bpool", bufs=1))
                psB = sb_.enter_context(tc.tile_pool(name="psB", bufs=1,
                                                     space="PSUM"))

                wo_t = [bpool.tile([DH, D], f32r, name=f"wo{h}")
                        for h in range(HPC)]
                for h in range(HPC):
                    nc.scalar.dma_start(out=wo_t[h], in_=wo_r[:, h, :])

                def emit_wo_group(qp_, outs_, g):
                    """Wo matmuls + output stream for m-pair g of q-chunk qp_."""
                    q0p = qp_ * 512
                    yps = [psB.tile([128, 512], f32, name=f"yp{j}",
                                    tag=f"yp{j}", bufs=1) for j in range(2)]
                    for h in range(HPC):
                        for j in range(2):
                            m = 2 * g + j
                            nc.tensor.matmul(
                                yps[j], wo_t[h][:, m * 128:(m + 1) * 128],
                                outs_[h][:],
                                start=(h == 0), stop=(h == HPC - 1))
                    for j in range(2):
                        m = 2 * g + j
                        ysb = bpool.tile([128, 512], f32, name="ysb",
                                         tag="ysb", bufs=8)
                        nc.vector.tensor_copy(out=ysb, in_=yps[j])
                        nc.sync.dma_start(
                            out=yt_d[m * 128:(m + 1) * 128, q0p:q0p + 512],
                            in_=ysb[:])

                prev = None  # (qc, outs) whose Wo work is still pending
                for qc in range(TC):
                    q0 = qc * 512
                    b = q0 // N
                    qb = q0 - b * N
                    outs = []
                    for h in range(HPC):
                        ap = psB.tile([128, 512], f32, name="ap",
                                      tag="ap", bufs=1)
                        folds = []
                        exs = []
                        for i in range(KP):
                            dp = psB.tile([128, 1024], f32, name="dp",
                                          tag="dp", bufs=2)
                            for half in range(2):
                                kt = 2 * i + half
                                nc.tensor.matmul(
                                    dp[:, half * 512:(half + 1) * 512],
                                    krt[h][b][:, kt * 128:(kt + 1) * 128],
                                    qrt[h][b][:, qb:qb + 512],
                                    start=True, stop=True)
                            ex = bpool.tile([128, 1024], bf16, name="ex",
                                            tag="ex", bufs=12)
                            nc.scalar.activation(
                                out=ex, in_=dp[:],
                                func=mybir.ActivationFunctionType.Exp)
                            exs.append(ex)
                            # attn@v matmuls trail the exp by one i so the
                            # PE never waits on ACT latency
                            if i > 0:
                                exp_ = exs[i - 1]
                                for half in range(2):
                                    kt = 2 * (i - 1) + half
                                    nc.tensor.matmul(
                                        ap, vnat[h][b][:, kt, :],
                                        exp_[:, half * 512:(half + 1) * 512],
                                        start=(kt == 0), stop=False)
                                fold = bpool.tile([128, 512], bf16,
                                                  name="fold", tag="fold",
                                                  bufs=12)
                                nc.vector.tensor_add(
                                    fold, exp_[:, 0:512], exp_[:, 512:1024])
                                folds.append(fold)
                            if prev is not None and i % 2 == 1:
                                emit_wo_group(prev[0], prev[1],
                                              h * (KP // 2) + i // 2)
                        for half in range(2):
                            kt = 2 * (KP - 1) + half
                            nc.tensor.matmul(
                                ap, vnat[h][b][:, kt, :],
                                exs[-1][:, half * 512:(half + 1) * 512],
                                start=False, stop=(half == 1))
                        fold = bpool.tile([128, 512], bf16, name="fold",
                                          tag="fold", bufs=12)
                        nc.vector.tensor_add(
                            fold, exs[-1][:, 0:512], exs[-1][:, 512:1024])
                        folds.append(fold)
                        # denominator tree on Pool: 8 folds -> 2 tiles, then
                        # two accumulating ones-matmuls do the final
                        # 128-partition reduce + broadcast on the PE
                        lvl = []
                        for j in range(4):
                            p = bpool.tile([128, 512], bf16, name="tp",
                                           tag="tp", bufs=8)
                            nc.gpsimd.tensor_add(p, folds[2 * j],
                                                 folds[2 * j + 1])
                            lvl.append(p)
                        fin = []
                        for j in range(2):
                            p = bpool.tile([128, 512], bf16, name="tq",
                                           tag="tq", bufs=4)
                            nc.gpsimd.tensor_add(p, lvl[2 * j],
                                                 lvl[2 * j + 1])
                            fin.append(p)
                        sp = psB.tile([128, 512], f32, name="sp",
                                      tag="sp", bufs=1)
                        nc.tensor.matmul(sp, onesb[:], fin[0][:],
                                         start=True, stop=False)
                        nc.tensor.matmul(sp, onesb[:], fin[1][:],
                                         start=False, stop=True)
                        rcp = bpool.tile([128, 512], f32, name="rcp",
                                         tag="rcp", bufs=2)
                        nc.vector.reciprocal_approx_fast(out=rcp[:],
                                                         in_=sp[:])
                        ot = bpool.tile([128, 512], f32r, name=f"ot{h}",
                                        tag=f"ot{h}", bufs=2)
                        nc.vector.tensor_mul(ot[:], ap[:], rcp[:])
                        outs.append(ot)
                    prev = (qc, outs)
                # flush the last q-chunk's Wo work
                for g in range(8):
                    emit_wo_group(prev[0], prev[1], g)

    nc.compile()
    return nc


def _host_prep(x, rotary_emb, Wq, Wkv, Wo, bo):
    import ml_dtypes
    bf16 = ml_dtypes.bfloat16

    x = np.asarray(x, dtype=np.float32)
    rotary_emb = np.asarray(rotary_emb, dtype=np.float32)
    Wq = np.asarray(Wq, dtype=np.float32)
    Wkv = np.asarray(Wkv, dtype=np.float32)
    Wo = np.asarray(Wo, dtype=np.float32)

    xt = np.ascontiguousarray(x.reshape(TOK, D).T.astype(bf16))
    cost = np.ascontiguousarray(np.cos(rotary_emb).T)
    sgn = np.where(np.arange(DH) % 2 == 0, -1.0, 1.0).astype(np.float32)
    sint = np.ascontiguousarray((np.sin(rotary_emb) * sgn).T)

    in_maps = []
    for c in range(NCORES):
        sl = slice(c * INC, (c + 1) * INC)
        in_maps.append({
            "xt": xt,
            "wq": np.ascontiguousarray((Wq[:, sl] * SCALE).astype(bf16)),
            "wk": np.ascontiguousarray(Wkv[:, sl].astype(bf16)),
            "wv": np.ascontiguousarray(
                Wkv[:, D + c * INC:D + (c + 1) * INC].astype(bf16)),
            "wo": np.ascontiguousarray(Wo[sl, :]),
            "cost": cost,
            "sint": sint,
        })
    return in_maps


def _get_nc():
    if "nc" not in _CACHE:
        _CACHE["nc"] = _build()
    return _CACHE["nc"]


def run_sharded(in_maps, trace=False, tmpdir=None):
    from concourse.bass_utils import run_bass_kernel_spmd
    nc = _get_nc()
    return run_bass_kernel_spmd(nc, in_maps, list(range(NCORES)),
                                trace=trace, tmpdir=tmpdir)


def _finish(results, bo):
    yt = results[0]["yt"].astype(np.float64)
    for c in range(1, NCORES):
        yt += results[c]["yt"]
    y = np.ascontiguousarray(yt.T).reshape(B, N, D)
    y = y + np.asarray(bo, dtype=np.float64)[None, None, :]
    return y.astype(np.float32)


def kernel(x, rotary_emb, Wq, Wkv, Wo, bo):
    in_maps = _host_prep(x, rotary_emb, Wq, Wkv, Wo, bo)
    res = run_sharded(in_maps)
    return _finish(res.results, bo)


# revision 6
# speedup vs baseline: 1.1012x; 1.0479x over previous
"""TRN2 Bass kernel for nn_Attention_24309514895857.

Multi-head attention (16 heads, dim_head 128, d_model 2048, b=2, n=2048) with
rotary embedding, sharded tensor-parallel over 8 NeuronCores: 2 heads per core.
Each core computes q/k/v projections for its heads, rotary, softmax attention,
and its partial contribution to the output projection (row-parallel Wo). The
host sums the 8 partials (the row-parallel unshard) and adds the bias.

Perf structure (v2):
- Projections run in bf16 (x and Wq/Wk/Wv are host-cast): halves the phase-A
  HBM stream so the PE never starves at t=0 (keeps HAM at 2.4GHz). PSUM
  accumulation is fp32, so everything downstream is unchanged.
- Softmax denominator: instead of 16 accumulating ones-matmuls per
  (q-chunk, head) on the PE (54us of PE time), the exp'd score tiles are
  pairwise-reduced on DVE (self-folds, bf16 2x mode) + Pool (tree adds),
  leaving only TWO ones-matmuls per (q-chunk, head) for the final
  128-partition reduction + broadcast.
- exp runs on ACT over [128,1024] two-bank PSUM tiles (half the instruction
  overhead) and writes bf16; the attn@v matmuls consume the bf16 exps.
- The Wo matmuls for q-chunk qc are deferred and interleaved into the
  attention matmuls of qc+1, so the PE never waits on the softmax
  normalization (DVE reciprocal) latency.
- Output bias is applied on the host; PSUM->SBUF output copies run on DVE
  and stream out on the sync DMA queue.
"""

import numpy as np

HEADS = 16
DH = 128          # dim_head
D = 2048          # d_model
B = 2
N = 2048          # seq len
TOK = B * N       # 4096 flattened tokens
NCORES = 8
HPC = HEADS // NCORES   # 2 heads per core
INC = HPC * DH          # 256 inner cols per core
KCH = D // 128          # 16 model-dim chunks
TC = TOK // 512         # 8 token chunks of 512
KT = N // 128           # 16 k-token chunks of 128 per batch
KP = KT // 2            # 8 k-token chunk PAIRS (dp tiles span 2 chunks)
SCALE = DH ** -0.5

_CACHE = {}

# DVE stream_shuffle mask: swap adjacent lane pairs within each 32-lane group
SWAP_MASK = []
for _i in range(16):
    SWAP_MASK += [2 * _i + 1, 2 * _i]


def _build():
    import concourse.bacc as bacc
    import concourse.tile as tile
    from concourse import mybir

    f32 = mybir.dt.float32
    f32r = mybir.dt.float32r
    bf16 = mybir.dt.bfloat16

    nc = bacc.Bacc("TRN2", target_bir_lowering=False, debug=False,
                   num_devices=NCORES)

    xt_d = nc.dram_tensor("xt", [D, TOK], bf16, kind="ExternalInput").ap()
    wq_d = nc.dram_tensor("wq", [D, INC], bf16, kind="ExternalInput").ap()
    wk_d = nc.dram_tensor("wk", [D, INC], bf16, kind="ExternalInput").ap()
    wv_d = nc.dram_tensor("wv", [D, INC], bf16, kind="ExternalInput").ap()
    wo_d = nc.dram_tensor("wo", [INC, D], f32, kind="ExternalInput").ap()
    cos_d = nc.dram_tensor("cost", [DH, N], f32, kind="ExternalInput").ap()
    sin_d = nc.dram_tensor("sint", [DH, N], f32, kind="ExternalInput").ap()
    yt_d = nc.dram_tensor("yt", [D, TOK], f32, kind="ExternalOutput").ap()

    xt_r = xt_d.rearrange("(k p) t -> p k t", p=128)
    wq_r = wq_d.rearrange("(k p) j -> p k j", p=128)
    wk_r = wk_d.rearrange("(k p) j -> p k j", p=128)
    wv_r = wv_d.rearrange("(k p) j -> p k j", p=128)
    wo_r = wo_d.bitcast(f32r).rearrange("(h p) m -> p h m", p=128)

    with tile.TileContext(nc) as tc:
        import contextlib
        with contextlib.ExitStack() as stack:
            glob = stack.enter_context(tc.tile_pool(name="glob", bufs=1))
            qkv = stack.enter_context(tc.tile_pool(name="qkv", bufs=1))
            rot = stack.enter_context(tc.tile_pool(name="rot", bufs=1))

            onesf = glob.tile([128, 128], f32)
            nc.vector.memset(onesf, 1.0)
            ones = glob.tile([128, 128], f32r)
            nc.vector.tensor_copy(out=ones, in_=onesf)
            onesb = glob.tile([128, 128], bf16)
            nc.vector.tensor_copy(out=onesb, in_=onesf)

            # persistent per-head activations, split per batch so phase B's
            # reads only depend on that batch's phase-A writes
            qrt = [[qkv.tile([DH, N], bf16, name=f"qrt{h}b{b}")
                    for b in range(B)] for h in range(HPC)]
            krt = [[qkv.tile([DH, N], bf16, name=f"krt{h}b{b}")
                    for b in range(B)] for h in range(HPC)]
            vnat = [[qkv.tile([128, KT, DH], bf16, name=f"vnat{h}b{b}")
                     for b in range(B)] for h in range(HPC)]

            # HAM warm-up: free matmuls while the first DMAs land, so real
            # matmuls start at 2.4GHz instead of the cold 1.2GHz
            with tc.tile_pool(name="pwarm", bufs=1, space="PSUM") as pw:
                wps = pw.tile([128, 128], f32)
                NWARM = 56
                for i in range(NWARM):
                    nc.tensor.matmul(wps, ones[:], ones[:],
                                     start=(i == 0), stop=(i == NWARM - 1))

            # ---------------- Phase A: projections + rotary ----------------
            with contextlib.ExitStack() as sa:
                wpool = sa.enter_context(tc.tile_pool(name="wpool", bufs=1))
                apool = sa.enter_context(tc.tile_pool(name="apool", bufs=1))
                wq_t = [wpool.tile([128, INC], bf16, name=f"wq{k}")
                        for k in range(KCH)]
                wk_t = [wpool.tile([128, INC], bf16, name=f"wk{k}")
                        for k in range(KCH)]
                wv_t = [wpool.tile([128, INC], bf16, name=f"wv{k}")
                        for k in range(KCH)]
                psA = sa.enter_context(tc.tile_pool(name="psA", bufs=1,
                                                    space="PSUM"))
                cost = apool.tile([DH, N], f32)
                sint = apool.tile([DH, N], f32)
                costL = rot.tile([DH, 512], f32)
                sintL = rot.tile([DH, 512], f32)
                # weights + rotary tables on the scalar DMA queue in
                # consumption order; x tiles stream JIT on the sync queue
                for k in range(KCH):
                    nc.scalar.dma_start(out=wq_t[k], in_=wq_r[:, k, :])
                    nc.scalar.dma_start(out=wk_t[k], in_=wk_r[:, k, :])
                    nc.scalar.dma_start(out=wv_t[k], in_=wv_r[:, k, :])
                # rotary tables after all weights: rotary (DVE) tolerates the
                # latency, the projection matmuls don't
                nc.scalar.dma_start(out=cost, in_=cos_d)
                nc.scalar.dma_start(out=sint, in_=sin_d)

                for t in range(TC):
                    tok0 = t * 512
                    tb = tok0 // N
                    bo0 = tok0 - tb * N
                    if t == TC - 1:
                        lsl = slice((t % (N // 512)) * 512,
                                    (t % (N // 512)) * 512 + 512)
                        nc.scalar.copy(out=costL, in_=cost[:, lsl])
                        nc.scalar.copy(out=sintL, in_=sint[:, lsl])
                    nsl = slice((t % (N // 512)) * 512,
                                (t % (N // 512)) * 512 + 512)
                    qp = [psA.tile([128, 512], f32, name=f"qp{h}", tag=f"qp{h}")
                          for h in range(HPC)]
                    kp = [psA.tile([128, 512], f32, name=f"kp{h}", tag=f"kp{h}")
                          for h in range(HPC)]
                    vp = [psA.tile([128, INC], f32, name=f"vp{s}",
                                   tag=f"vp{s}", bufs=1) for s in range(4)]
                    for k in range(KCH):
                        xt = apool.tile([128, 512], bf16, name="xt",
                                        tag="xt", bufs=8)
                        nc.sync.dma_start(
                            out=xt, in_=xt_r[:, k, tok0:tok0 + 512])
                        for h in range(HPC):
                            nc.tensor.matmul(
                                qp[h], wq_t[k][:, h * DH:(h + 1) * DH], xt[:],
                                start=(k == 0), stop=(k == KCH - 1))
                            nc.tensor.matmul(
                                kp[h], wk_t[k][:, h * DH:(h + 1) * DH], xt[:],
                                start=(k == 0), stop=(k == KCH - 1))
                        for sub in range(4):
                            nc.tensor.matmul(
                                vp[sub],
                                xt[:, sub * 128:(sub + 1) * 128],
                                wv_t[k][:],
                                start=(k == 0), stop=(k == KCH - 1))
                    # v psum -> token-major bf16 SBUF on ACT (frees vp banks;
                    # keeps DVE free for rotary)
                    for sub in range(4):
                        chunk = (t % 4) * 4 + sub
                        for h in range(HPC):
                            nc.scalar.copy(
                                out=vnat[h][tb][:, chunk, :],
                                in_=vp[sub][:, h * DH:(h + 1) * DH])
                    # rotary for q and k of both heads (DVE): rotate_half's
                    # pair swap is one stream_shuffle; sign lives in sint
                    for h in range(HPC):
                        for (pp, dst) in ((qp[h], qrt[h][tb]),
                                          (kp[h], krt[h][tb])):
                            sb = rot.tile([128, 512], f32r, name="rsb",
                                          tag="rsb", bufs=4)
                            nc.scalar.copy(out=sb, in_=pp)
                            sbs = rot.tile([128, 512], f32, name="sbs",
                                           tag="sbs", bufs=2)
                            nc.vector.stream_shuffle(
                                out=sbs[:], in_=sb[:].bitcast(f32),
                                mask=SWAP_MASK)
                            cs = cost[:, nsl] if t < TC - 1 else costL[:]
                            sn = sint[:, nsl] if t < TC - 1 else sintL[:]
                            t1 = rot.tile([128, 512], f32, name="t1",
                                          tag="t1", bufs=3)
                            nc.vector.tensor_mul(
                                t1[:], sb[:].bitcast(f32), cs)
                            t2 = rot.tile([128, 512], f32, name="t2",
                                          tag="t2", bufs=3)
                            nc.vector.tensor_mul(t2[:], sbs[:], sn)
                            nc.vector.tensor_add(
                                dst[:, bo0:bo0 + 512], t1[:], t2[:])

            # ---------------- Phase B+C: attention + output proj -----------
            with contextlib.ExitStack() as sb_:
                bpool = sb_.enter_context(tc.tile_pool(name="bpool", bufs=1))
                psB = sb_.enter_context(tc.tile_pool(name="psB", bufs=1,
                                                     space="PSUM"))

                wo_t = [bpool.tile([DH, D], f32r, name=f"wo{h}")
                        for h in range(HPC)]
                for h in range(HPC):
                    nc.scalar.dma_start(out=wo_t[h], in_=wo_r[:, h, :])

                def emit_wo_group(qp_, outs_, g):
                    """Wo matmuls + output stream for m-pair g of q-chunk qp_."""
                    q0p = qp_ * 512
                    yps = [psB.tile([128, 512], f32, name=f"yp{j}",
                                    tag=f"yp{j}", bufs=1) for j in range(2)]
                    for h in range(HPC):
                        for j in range(2):
                            m = 2 * g + j
                            nc.tensor.matmul(
                                yps[j], wo_t[h][:, m * 128:(m + 1) * 128],
                                outs_[h][:],
                                start=(h == 0), stop=(h == HPC - 1))
                    for j in range(2):
                        m = 2 * g + j
                        ysb = bpool.tile([128, 512], f32, name="ysb",
                                         tag="ysb", bufs=8)
                        nc.vector.tensor_copy(out=ysb, in_=yps[j])
                        nc.sync.dma_start(
                            out=yt_d[m * 128:(m + 1) * 128, q0p:q0p + 512],
                            in_=ysb[:])

                TRAIL = 2  # attn@v trails exp by 2 tiles (ACT latency slack)
                prev = None  # (qc, outs) whose Wo work is still pending
                for qc in range(TC):
                    q0 = qc * 512
                    b = q0 // N
                    qb = q0 - b * N
                    outs = []
                    for h in range(HPC):
                        ap = psB.tile([128, 512], f32, name="ap",
                                      tag="ap", bufs=1)
                        folds = []
                        exs = []

                        def consume(i):
                            """ap matmuls + denominator fold for exp tile i."""
                            exp_ = exs[i]
                            for half in range(2):
                                kt = 2 * i + half
                                nc.tensor.matmul(
                                    ap, vnat[h][b][:, kt, :],
                                    exp_[:, half * 512:(half + 1) * 512],
                                    start=(kt == 0), stop=(kt == KT - 1))
                            fold = bpool.tile([128, 512], bf16, name="fold",
                                              tag="fold", bufs=12)
                            # 12 of 16 folds per q-chunk on DVE (2x bf16),
                            # 4 on Pool to balance the queues
                            eng = nc.gpsimd if i in (2, 5) else nc.vector
                            eng.tensor_add(
                                fold, exp_[:, 0:512], exp_[:, 512:1024])
                            folds.append(fold)

                        for i in range(KP):
                            dp = psB.tile([128, 1024], f32, name="dp",
                                          tag="dp", bufs=2)
                            for half in range(2):
                                kt = 2 * i + half
                                nc.tensor.matmul(
                                    dp[:, half * 512:(half + 1) * 512],
                                    krt[h][b][:, kt * 128:(kt + 1) * 128],
                                    qrt[h][b][:, qb:qb + 512],
                                    start=True, stop=True)
                            ex = bpool.tile([128, 1024], bf16, name="ex",
                                            tag="ex", bufs=12)
                            nc.scalar.activation(
                                out=ex, in_=dp[:],
                                func=mybir.ActivationFunctionType.Exp)
                            exs.append(ex)
                            if i >= TRAIL:
                                consume(i - TRAIL)
                            # Wo slots: h0 carries groups 0-3, h1 carries 4-5;
                            # groups 6-7 are emitted after the normalization
                            # chain so they cover its latency on the PE
                            if prev is not None and i % 2 == 1:
                                g = h * 4 + i // 2
                                if g < 6:
                                    emit_wo_group(prev[0], prev[1], g)
                        for i in range(KP - TRAIL, KP):
                            consume(i)
                        # denominator: 8 folds -> 4 on Pool, then 4
                        # accumulating ones-matmuls reduce over partitions
                        # and broadcast the sums
                        lvl = []
                        for j in range(4):
                            p = bpool.tile([128, 512], bf16, name="tp",
                                           tag="tp", bufs=8)
                            nc.gpsimd.tensor_add(p, folds[2 * j],
                                                 folds[2 * j + 1])
                            lvl.append(p)
                        sp = psB.tile([128, 512], f32, name="sp",
                                      tag="sp", bufs=1)
                        for j in range(4):
                            nc.tensor.matmul(sp, onesb[:], lvl[j][:],
                                             start=(j == 0), stop=(j == 3))
                        rcp = bpool.tile([128, 512], f32, name="rcp",
                                         tag="rcp", bufs=2)
                        nc.vector.reciprocal_approx_fast(out=rcp[:],
                                                         in_=sp[:])
                        ot = bpool.tile([128, 512], f32r, name=f"ot{h}",
                                        tag=f"ot{h}", bufs=2)
                        nc.vector.tensor_mul(ot[:], ap[:], rcp[:])
                        outs.append(ot)
                        if prev is not None and h == HPC - 1:
                            # the last two Wo groups run right after the
                            # normalization chain is emitted: the PE chews
                            # them while DVE finishes rcp/ot for this chunk
                            emit_wo_group(prev[0], prev[1], 6)
                            emit_wo_group(prev[0], prev[1], 7)
                    prev = (qc, outs)
                # flush the last q-chunk's Wo work
                for g in range(8):
                    emit_wo_group(prev[0], prev[1], g)

    nc.compile()
    return nc


def _host_prep(x, rotary_emb, Wq, Wkv, Wo, bo):
    import ml_dtypes
    bf16 = ml_dtypes.bfloat16

    x = np.asarray(x, dtype=np.float32)
    rotary_emb = np.asarray(rotary_emb, dtype=np.float32)
    Wq = np.asarray(Wq, dtype=np.float32)
    Wkv = np.asarray(Wkv, dtype=np.float32)
    Wo = np.asarray(Wo, dtype=np.float32)

    xt = np.ascontiguousarray(x.reshape(TOK, D).T.astype(bf16))
    cost = np.ascontiguousarray(np.cos(rotary_emb).T)
    sgn = np.where(np.arange(DH) % 2 == 0, -1.0, 1.0).astype(np.float32)
    sint = np.ascontiguousarray((np.sin(rotary_emb) * sgn).T)

    in_maps = []
    for c in range(NCORES):
        sl = slice(c * INC, (c + 1) * INC)
        in_maps.append({
            "xt": xt,
            "wq": np.ascontiguousarray((Wq[:, sl] * SCALE).astype(bf16)),
            "wk": np.ascontiguousarray(Wkv[:, sl].astype(bf16)),
            "wv": np.ascontiguousarray(
                Wkv[:, D + c * INC:D + (c + 1) * INC].astype(bf16)),
            "wo": np.ascontiguousarray(Wo[sl, :]),
            "cost": cost,
            "sint": sint,
        })
    return in_maps


def _get_nc():
    if "nc" not in _CACHE:
        _CACHE["nc"] = _build()
    return _CACHE["nc"]


def run_sharded(in_maps, trace=False, tmpdir=None):
    from concourse.bass_utils import run_bass_kernel_spmd
    nc = _get_nc()
    return run_bass_kernel_spmd(nc, in_maps, list(range(NCORES)),
                                trace=trace, tmpdir=tmpdir)


def _finish(results, bo):
    yt = results[0]["yt"].astype(np.float64)
    for c in range(1, NCORES):
        yt += results[c]["yt"]
    y = np.ascontiguousarray(yt.T).reshape(B, N, D)
    y = y + np.asarray(bo, dtype=np.float64)[None, None, :]
    return y.astype(np.float32)


def kernel(x, rotary_emb, Wq, Wkv, Wo, bo):
    in_maps = _host_prep(x, rotary_emb, Wq, Wkv, Wo, bo)
    res = run_sharded(in_maps)
    return _finish(res.results, bo)
